# revision 7
# baseline (speedup 1.0000x reference)
"""Multi-head attention (B=4, L=2048, E=1024, H=16, DK=64) on 8 TRN2 cores.

Sharding: core c -> (batch b = c//2, head-group g = c%2 of 8 heads).
v2: early attention start (per-j qkv triples interleaved with attention
blocks), per-head split exp (pipelines next scores under the second exp
half), fp8 DoubleRow AV matmuls (2 key-blocks per matmul, ones column
kept for rowsums), fc interleaved into the next half's blocks, RS#2
split into two 512-token collectives so only the last ~1 MiB sits in
the tail.

Self-contained: hardcodes all shapes; requires only the concourse stack.
"""

import numpy as np
import ml_dtypes

try:
    import axon_prof

    axon_prof.install()
except Exception:
    pass

import concourse.mybir as mybir
import concourse.tile as tile
from concourse import bacc
from concourse import bass_utils

B, L, E = 4, 2048, 1024
H, DK = 16, 64
H8 = 8                      # heads per core
F = H8 * 3 * DK             # qkv features per core = 1536
FO = H8 * DK                # attn-out features per core = 512
NCORES = 8
LHALF = L // 2

f32 = mybir.dt.float32
bf16 = mybir.dt.bfloat16
f8 = mybir.dt.float8e4
Exp = mybir.ActivationFunctionType.Exp
MUL = mybir.AluOpType.mult
ADD = mybir.AluOpType.add
DR = mybir.MatmulPerfMode.DoubleRow

_CACHE = {}


def build_nc():
    nc = bacc.Bacc("TRN2", target_bir_lowering=False, debug=False, num_devices=NCORES)

    # x arrives already transposed (host-side) so no xbar transpose is needed
    x = nc.dram_tensor("x", [E, L], bf16, kind="ExternalInput")
    w_qkv = nc.dram_tensor("w_qkv", [E, F], bf16, kind="ExternalInput")
    b_qkv = nc.dram_tensor("b_qkv", [128, 12], f32, kind="ExternalInput")
    w_fc = nc.dram_tensor("w_fc", [FO, E], bf16, kind="ExternalInput")
    b_fc = nc.dram_tensor("b_fc", [1, E], f32, kind="ExternalInput")
    out = nc.dram_tensor("out", [LHALF, E], f32, kind="ExternalOutput")

    with tile.TileContext(nc) as tc:
        with (
            tc.tile_pool(name="persist", bufs=1) as pp,
            tc.tile_pool(name="work", bufs=3) as wp,
            tc.tile_pool(name="ptp", bufs=6) as ptpool,
            tc.tile_pool(name="stage", bufs=1) as sp,
            tc.tile_pool(name="ys", bufs=3) as yp_pool,
            tc.tile_pool(name="dram", bufs=1, space="DRAM") as dram,
        ):
            # ---- persistent SBUF ----
            xT = pp.tile([128, 8, L], bf16, tag="xT")          # X^T  4 MiB
            wq = pp.tile([128, 8, F], bf16, tag="wq")          # 3 MiB
            bq = pp.tile([128, 12], f32, tag="bq")
            wfc = pp.tile([128, 4, E], bf16, tag="wfc")        # 1 MiB
            bias = pp.tile([128, E], f32, tag="bias")          # 0.5 MiB
            qt = pp.tile([128, 4, L], bf16, tag="qt")          # Q^T 2 MiB
            kt = pp.tile([128, 4, L], bf16, tag="kt")          # K^T 2 MiB
            # V natural layout, 80-elem stride; col 64 holds the ones column
            # so AV matmuls with lhsT [V|1] (M=65) produce rowsums for free
            v = pp.tile([128, H8, 16, 80], bf16, tag="v")      # 2.5 MiB
            v8 = pp.tile([128, H8, 16, 80], f8, tag="v8")      # fp8 copy
            onT = pp.tile([128, 4, L], bf16, tag="onT")        # attn out^T 2 MiB

            # ---- input DMAs: xT on sync queue, weights on vector queue ----
            for e in range(8):
                nc.sync.dma_start(xT[:, e, :], x[e * 128 : (e + 1) * 128, :])
            nc.gpsimd.dma_start(bq[:], b_qkv[:])
            nc.gpsimd.dma_start(wfc[:], w_fc.rearrange("(c p) e -> p c e", p=128))
            bfc_row = pp.tile([1, E], f32, tag="bfc_row")
            nc.gpsimd.dma_start(bfc_row[:], b_fc[:])
            nc.gpsimd.partition_broadcast(bias[:], bfc_row[:])
            nc.vector.memset(v[:, :, :, 64:65], 1.0)
            # wq per-ft column slices, in consumption order
            FT_ORDER = []
            for j in range(4):
                FT_ORDER += [j, 4 + j, 8 + j]
            for ft in FT_ORDER:
                nc.scalar.dma_start(
                    wq[:, :, ft * 128 : (ft + 1) * 128],
                    w_qkv.rearrange("(c p) f -> p c f", p=128)[
                        :, :, ft * 128 : (ft + 1) * 128
                    ],
                )
            # preload the exp table set during the DMA phase
            warm = sp.tile([128, 16], f32, tag="warm")
            nc.vector.memset(warm[:], 0.0)
            nc.scalar.activation(warm[:], warm[:], Exp)

            # ---- attention/qkv psum pools (8 banks total: st 4, av 4) ----
            with (
                tc.tile_pool(name="pst", bufs=1, space="PSUM") as pst_pool,
                tc.tile_pool(name="psav0", bufs=1, space="PSUM") as psav0_pool,
                tc.tile_pool(name="psav1", bufs=1, space="PSUM") as psav1_pool,
            ):

                def qkv_tile(ft):
                    # qkv^T = W_shard.T @ X^T (+bias) in two [128,1024]
                    # halves that borrow the av psum slots; the slot WAR
                    # defers the concurrent block's AV (pt bufs absorb it)
                    ps_a = psav0_pool.tile([128, LHALF], f32, tag="av0", name="ps_a")
                    ps_b = psav1_pool.tile([128, LHALF], f32, tag="av1", name="ps_b")
                    for kc in range(8):
                        lhsT = wq[:, kc, ft * 128 : (ft + 1) * 128]
                        for tb in range(4):
                            ps = ps_a if tb < 2 else ps_b
                            c0 = (tb % 2) * 512
                            nc.tensor.matmul(
                                ps[:, c0 : c0 + 512],
                                lhsT,
                                xT[:, kc, tb * 512 : (tb + 1) * 512],
                                start=(kc == 0),
                                stop=(kc == 7),
                            )
                    if ft < 4:
                        dst = qt[:, ft, :]
                    elif ft < 8:
                        dst = kt[:, ft - 4, :]
                    else:
                        dst = None
                    if dst is not None:
                        nc.vector.tensor_scalar_add(
                            dst[:, 0:1024], ps_a[:], bq[:, ft : ft + 1]
                        )
                        nc.vector.tensor_scalar_add(
                            dst[:, 1024:2048], ps_b[:], bq[:, ft : ft + 1]
                        )
                    else:
                        j = ft - 8
                        vt = sp.tile([128, L], bf16, tag="vt")
                        nc.vector.tensor_scalar_add(
                            vt[:, 0:1024], ps_a[:], bq[:, ft : ft + 1]
                        )
                        nc.vector.tensor_scalar_add(
                            vt[:, 1024:2048], ps_b[:], bq[:, ft : ft + 1]
                        )
                        # V^T -> V (token-major) via xbar transpose, per head
                        for hh in range(2):
                            h = 2 * j + hh
                            nc.sync.dma_start_transpose(
                                v[:, h, :, 0:DK], vt[hh * 64 : hh * 64 + 64, :]
                            )
                        # fp8 copy for DoubleRow AV (includes the ones col)
                        nc.vector.tensor_copy(
                            v8[:, 2 * j : 2 * j + 2, :, :],
                            v[:, 2 * j : 2 * j + 2, :, :],
                        )

                # ---- attention block ----
                rs_in = [
                    dram.tile([LHALF, E], bf16, name=f"rs_in{i}", tag=f"rs_in{i}")
                    for i in range(2)
                ]
                rs_out1 = dram.tile([LHALF // 2, E], bf16, name="rs_out1", tag="rs_out1")
                rs_out2 = [
                    dram.tile(
                        [LHALF // 4, E], bf16, name=f"rs_out2{i}", tag=f"rs_out2{i}"
                    )
                    for i in range(2)
                ]
                PAIRS = [[0, 1], [2, 3], [4, 5], [6, 7]]
                if True:

                    def attn_block(qb, j):
                        av0 = psav0_pool.tile([128, LHALF], f32, tag="av0")
                        av1 = psav1_pool.tile([128, LHALF], f32, tag="av1")
                        ptps = {}

                        def emit_st(kk):
                            st = pst_pool.tile([128, L], f32, tag="st", name="st")
                            for u in range(2):
                                q0 = qb * LHALF + u * 512
                                nc.tensor.matmul(
                                    st[:, u * 512 : (u + 1) * 512],
                                    kt[0:64, j, kk * 128 : (kk + 1) * 128],
                                    qt[0:64, j, q0 : q0 + 512],
                                    start=True,
                                    stop=True,
                                )
                                nc.tensor.matmul(
                                    st[:, 1024 + u * 512 : 1024 + (u + 1) * 512],
                                    kt[64:128, j, kk * 128 : (kk + 1) * 128],
                                    qt[64:128, j, q0 : q0 + 512],
                                    start=True,
                                    stop=True,
                                )
                            if kk % 2 == 0:
                                ptps[kk // 2] = ptpool.tile(
                                    [128, 2, L], f8, tag="pt", name="ptp"
                                )
                            ptp = ptps[kk // 2]
                            # per-head halves: frees st banks 0-1 while the
                            # second exp still runs -> next scores overlap
                            nc.scalar.activation(
                                ptp[:, kk % 2, 0:1024], st[:, 0:1024], Exp,
                                scale=0.125,
                            )
                            nc.scalar.activation(
                                ptp[:, kk % 2, 1024:2048], st[:, 1024:2048], Exp,
                                scale=0.125,
                            )

                        def emit_av(p):
                            ptp = ptps.pop(p)
                            first, last = p == 0, p == 7
                            for h in range(2):
                                av = av0 if h == 0 else av1
                                for u in range(2):
                                    off = h * 1024 + u * 512
                                    nc.tensor.matmul(
                                        av[0:65, u * 512 : (u + 1) * 512],
                                        v8[:, 2 * j + h, 2 * p : 2 * p + 2, 0:65],
                                        ptp[:, :, off : off + 512],
                                        start=first,
                                        stop=last,
                                        perf_mode=DR,
                                    )

                        for kk in range(16):
                            emit_st(kk)
                            if kk % 2 == 1 and kk > 1:
                                emit_av(kk // 2 - 1)
                        emit_av(7)

                        # early evict (frees av psum): unnormalized out^T.
                        # av1 rows 0:64 must land on partitions 64:128 -> DMA.
                        qsl = slice(qb * LHALF, (qb + 1) * LHALF)
                        nc.vector.tensor_copy(onT[0:64, j, qsl], av0[0:64, :])
                        tmp = wp.tile([64, LHALF], bf16, tag="tmp")
                        nc.vector.tensor_copy(tmp[:], av1[0:64, :])
                        srs = sp.tile([128, 2 * L], f32, tag="srs")
                        nc.vector.tensor_copy(srs[64:65, 0:1024], av0[64:65, :])
                        nc.vector.tensor_copy(srs[64:65, 1024:2048], av1[64:65, :])
                        # deferred normalization (overlaps the next block):
                        # srs cols 0:2048 = sums row, 2048:4096 = broadcast
                        nc.sync.dma_start(onT[64:128, j, qsl], tmp[:])
                        nc.sync.dma_start(srs[0:1, 0:2048], srs[64:65, 0:2048])
                        nc.gpsimd.partition_broadcast(
                            srs[:, 2048:4096], srs[0:1, 0:2048]
                        )
                        nc.vector.reciprocal_approx_fast(
                            srs[:, 2048:4096], srs[:, 2048:4096]
                        )
                        nc.vector.tensor_tensor(
                            onT[0:64, j, qsl], onT[0:64, j, qsl],
                            srs[0:64, 2048:3072], op=MUL,
                        )
                        nc.vector.tensor_tensor(
                            onT[64:128, j, qsl], onT[64:128, j, qsl],
                            srs[64:128, 3072:4096], op=MUL,
                        )

                    def fc_chunks(qb, t8s):
                        # fc for token chunks; psum slots borrowed from av pools
                        for t8 in t8s:
                            t = qb * 8 + t8
                            pool = psav0_pool if t8 % 2 == 0 else psav1_pool
                            tag = "av0" if t8 % 2 == 0 else "av1"
                            yp = pool.tile([128, E], f32, tag=tag)
                            for c in range(4):
                                lhsT = onT[:, c, t * 128 : (t + 1) * 128]
                                for e2 in range(2):
                                    nc.tensor.matmul(
                                        yp[:, e2 * 512 : (e2 + 1) * 512],
                                        lhsT,
                                        wfc[:, c, e2 * 512 : (e2 + 1) * 512],
                                        start=(c == 0),
                                        stop=(c == 3),
                                    )
                            ys = yp_pool.tile([128, E], bf16, tag="ys")
                            nc.vector.tensor_tensor(ys[:], yp[:], bias[:], op=ADD)
                            nc.sync.dma_start(
                                rs_in[qb][t8 * 128 : (t8 + 1) * 128, :], ys[:]
                            )

                    # phase 1 + first-half attention, interleaved per j
                    for j in range(4):
                        qkv_tile(j)
                        qkv_tile(4 + j)
                        qkv_tile(8 + j)
                        attn_block(0, j)
                    # second half; fc/collectives fill PE slack
                    attn_block(1, 0)
                    fc_chunks(0, range(8))
                    nc.gpsimd.collective_compute(
                        "ReduceScatter", ADD, replica_groups=PAIRS,
                        ins=[rs_in[0].opt()], outs=[rs_out1.opt()],
                    )
                    attn_block(1, 1)
                    nc.gpsimd.dma_start(out[0 : LHALF // 2, :], rs_out1[:])
                    attn_block(1, 2)
                    attn_block(1, 3)
                    fc_chunks(1, range(4))
                    nc.gpsimd.collective_compute(
                        "ReduceScatter", ADD, replica_groups=PAIRS,
                        ins=[rs_in[1][0 : LHALF // 2, :].opt()],
                        outs=[rs_out2[0].opt()],
                    )
                    fc_chunks(1, range(4, 8))
                    nc.gpsimd.dma_start(
                        out[LHALF // 2 : 3 * LHALF // 4, :], rs_out2[0][:]
                    )
                    nc.gpsimd.collective_compute(
                        "ReduceScatter", ADD, replica_groups=PAIRS,
                        ins=[rs_in[1][LHALF // 2 :, :].opt()],
                        outs=[rs_out2[1].opt()],
                    )
                    nc.gpsimd.dma_start(
                        out[3 * LHALF // 4 :, :], rs_out2[1][:]
                    )

    nc.finalize()
    return nc


def _prep_inputs(X, W_qkv, b_qkv, W_fc, b_fc):
    """Host-side shard + permute + cast. Returns in_maps for 8 cores."""
    X = np.asarray(X, dtype=np.float32)
    W_qkv = np.asarray(W_qkv, dtype=np.float32)
    b_qkv = np.asarray(b_qkv, dtype=np.float32)
    W_fc = np.asarray(W_fc, dtype=np.float32)
    b_fc = np.asarray(b_fc, dtype=np.float32)

    in_maps = []
    bfc_half = (0.5 * b_fc).astype(np.float32).reshape(1, E)
    for c in range(NCORES):
        b, g = divmod(c, 2)
        heads = np.arange(g * H8, (g + 1) * H8)
        # column order: all Q feats (head-major), then K, then V
        cols = np.concatenate(
            [
                np.concatenate([h * 3 * DK + off + np.arange(DK) for h in heads])
                for off in (0, DK, 2 * DK)
            ]
        )
        wq_sh = W_qkv[:, cols].astype(ml_dtypes.bfloat16)
        bq_sh = b_qkv[cols].astype(np.float32).reshape(12, 128).T.copy()
        wfc_sh = W_fc[g * FO : (g + 1) * FO, :].astype(ml_dtypes.bfloat16)
        in_maps.append(
            {
                "x": np.ascontiguousarray(X[b].T).astype(ml_dtypes.bfloat16),
                "w_qkv": wq_sh,
                "b_qkv": np.ascontiguousarray(bq_sh),
                "w_fc": wfc_sh,
                "b_fc": bfc_half,
            }
        )
    return in_maps


def run_kernel(inputs, trace=False):
    if "nc" not in _CACHE:
        _CACHE["nc"] = build_nc()
    nc = _CACHE["nc"]
    in_maps = _prep_inputs(**inputs)
    res = bass_utils.run_bass_kernel_spmd(
        nc, in_maps, core_ids=list(range(NCORES)), trace=trace
    )
    Y = np.empty((B, L, E), dtype=np.float32)
    Q2 = LHALF // 2
    Q4 = LHALF // 4
    for c in range(NCORES):
        b, g = divmod(c, 2)
        o = res.results[c]["out"]
        # RS#1 scattered tokens [0:1024]; RS#2a [1024:1536]; RS#2b [1536:2048]
        Y[b, g * Q2 : (g + 1) * Q2, :] = o[0:Q2]
        Y[b, LHALF + g * Q4 : LHALF + (g + 1) * Q4, :] = o[Q2 : Q2 + Q4]
        Y[b, LHALF + Q2 + g * Q4 : LHALF + Q2 + (g + 1) * Q4, :] = o[Q2 + Q4 :]
    return Y, res


def kernel(X, W_qkv, b_qkv, W_fc, b_fc):
    Y, _ = run_kernel(
        dict(X=X, W_qkv=W_qkv, b_qkv=b_qkv, W_fc=W_fc, b_fc=b_fc), trace=False
    )
    return Y


if __name__ == "__main__":
    build_nc()
    print("kernel v2 compiled OK")


# revision 13
# speedup vs baseline: 1.2753x; 1.2753x over previous
"""Multi-head attention (B=4, L=2048, E=1024, H=16, DK=64) on 8 TRN2 cores.

Sharding: core c -> (batch b = c//2, head-group g = c%2 of 8 heads).
v4 schedule: first half = 512-query blocks (st ping-pong 2x[128,1024]
bufs=2 = 4 banks, av 2x[65,512] = 2 banks, dedicated 2-bank qkv psum
pool) so qkv projection tiles overlap the exp stream; second half =
1024-query blocks with per-head st half-tiles (true exp ping-pong) and
fc/collectives filling PE slack. exp runs back-to-back on ScalarE.

Self-contained: hardcodes all shapes; requires only the concourse stack.
"""

import numpy as np
import ml_dtypes

try:
    import axon_prof

    axon_prof.install()
except Exception:
    pass

import concourse.mybir as mybir
import concourse.tile as tile
from concourse import bacc
from concourse import bass_utils

B, L, E = 4, 2048, 1024
H, DK = 16, 64
H8 = 8                      # heads per core
F = H8 * 3 * DK             # qkv features per core = 1536
FO = H8 * DK                # attn-out features per core = 512
NCORES = 8
LHALF = L // 2

f32 = mybir.dt.float32
bf16 = mybir.dt.bfloat16
Exp = mybir.ActivationFunctionType.Exp
MUL = mybir.AluOpType.mult
ADD = mybir.AluOpType.add

_CACHE = {}


def build_nc():
    nc = bacc.Bacc("TRN2", target_bir_lowering=False, debug=False, num_devices=NCORES)

    x = nc.dram_tensor("x", [E, L], bf16, kind="ExternalInput")
    w_qkv = nc.dram_tensor("w_qkv", [E, F], bf16, kind="ExternalInput")
    b_qkv = nc.dram_tensor("b_qkv", [128, 12], f32, kind="ExternalInput")
    w_fc = nc.dram_tensor("w_fc", [FO, E], bf16, kind="ExternalInput")
    b_fc = nc.dram_tensor("b_fc", [1, E], f32, kind="ExternalInput")
    out = nc.dram_tensor("out", [LHALF, E], f32, kind="ExternalOutput")

    with tile.TileContext(nc) as tc:
        with (
            tc.tile_pool(name="persist", bufs=1) as pp,
            tc.tile_pool(name="work", bufs=3) as wp,
            tc.tile_pool(name="ptp", bufs=6) as ptpool,
            tc.tile_pool(name="stage", bufs=2) as sp,
            tc.tile_pool(name="srsp", bufs=1) as srsp,
            tc.tile_pool(name="ys", bufs=3) as yp_pool,
            tc.tile_pool(name="dram", bufs=1, space="DRAM") as dram,
        ):
            # ---- persistent SBUF ----
            xT = pp.tile([128, 8, L], bf16, tag="xT")          # X^T  4 MiB
            wq = pp.tile([128, 8, F], bf16, tag="wq")          # 3 MiB
            bq = pp.tile([128, 12], f32, tag="bq")
            wfc = pp.tile([128, 4, E], bf16, tag="wfc")        # 1 MiB
            bias = pp.tile([128, E], f32, tag="bias")          # 0.5 MiB
            qt = pp.tile([128, 4, L], bf16, tag="qt")          # Q^T 2 MiB
            kt = pp.tile([128, 4, L], bf16, tag="kt")          # K^T 2 MiB
            v = pp.tile([128, H8, 16, 80], bf16, tag="v")      # 2.5 MiB
            onT = pp.tile([128, 4, L], bf16, tag="onT")        # attn out^T 2 MiB

            # ---- input DMAs: xT on sync queue, wq slices on scalar queue
            for e in range(8):
                nc.sync.dma_start(xT[:, e, :], x[e * 128 : (e + 1) * 128, :])
            nc.gpsimd.dma_start(bq[:], b_qkv[:])
            nc.gpsimd.dma_start(wfc[:], w_fc.rearrange("(c p) e -> p c e", p=128))
            bfc_row = pp.tile([1, E], f32, tag="bfc_row")
            nc.gpsimd.dma_start(bfc_row[:], b_fc[:])
            nc.gpsimd.partition_broadcast(bias[:], bfc_row[:])
            nc.vector.memset(v[:, :, :, 64:65], 1.0)
            FT_ORDER = []
            for j in range(4):
                FT_ORDER += [j, 4 + j, 8 + j]
            for ft in FT_ORDER:
                nc.scalar.dma_start(
                    wq[:, :, ft * 128 : (ft + 1) * 128],
                    w_qkv.rearrange("(c p) f -> p c f", p=128)[
                        :, :, ft * 128 : (ft + 1) * 128
                    ],
                )
            # preload the exp table set during the DMA phase
            warm = pp.tile([128, 16], f32, tag="warm")
            nc.vector.memset(warm[:], 0.0)
            nc.scalar.activation(warm[:], warm[:], Exp)

            rs_in = [
                dram.tile([LHALF, E], bf16, name=f"rs_in{i}", tag=f"rs_in{i}")
                for i in range(2)
            ]
            rs_out = [
                dram.tile([LHALF // 2, E], bf16, name=f"rs_out{i}", tag=f"rs_out{i}")
                for i in range(2)
            ]
            PAIRS = [[0, 1], [2, 3], [4, 5], [6, 7]]

            def evict_norm(av0, av1, j, q0, qn):
                # unnormalized out^T + deferred normalization
                qsl = slice(q0, q0 + qn)
                nc.vector.tensor_copy(onT[0:64, j, qsl], av0[0:64, :])
                tmp = wp.tile([64, 1024], bf16, tag="tmp", name="tmp")
                nc.vector.tensor_copy(tmp[0:64, 0:qn], av1[0:64, :])
                srs = srsp.tile([128, 4096], f32, tag="srs", name="srs")
                nc.vector.tensor_copy(srs[64:65, 0:qn], av0[64:65, :])
                nc.vector.tensor_copy(srs[64:65, qn : 2 * qn], av1[64:65, :])
                nc.sync.dma_start(onT[64:128, j, qsl], tmp[0:64, 0:qn])
                nc.sync.dma_start(srs[0:1, 0 : 2 * qn], srs[64:65, 0 : 2 * qn])
                nc.gpsimd.partition_broadcast(
                    srs[:, 2048 : 2048 + 2 * qn], srs[0:1, 0 : 2 * qn]
                )
                nc.vector.reciprocal_approx_fast(
                    srs[:, 2048 : 2048 + 2 * qn], srs[:, 2048 : 2048 + 2 * qn]
                )
                nc.vector.tensor_tensor(
                    onT[0:64, j, qsl], onT[0:64, j, qsl],
                    srs[0:64, 2048 : 2048 + qn], op=MUL,
                )
                nc.vector.tensor_tensor(
                    onT[64:128, j, qsl], onT[64:128, j, qsl],
                    srs[64:128, 2048 + qn : 2048 + 2 * qn], op=MUL,
                )

            # ================= first half: 512-query blocks =================
            with (
                tc.tile_pool(name="pstA", bufs=2, space="PSUM") as pstA_pool,
                tc.tile_pool(name="psavA0", bufs=1, space="PSUM") as psavA0,
                tc.tile_pool(name="psavA1", bufs=1, space="PSUM") as psavA1,
                tc.tile_pool(name="psqkv", bufs=2, space="PSUM") as psqkv,
            ):

                def qkv_tile(ft):
                    for tb in range(4):
                        ps = psqkv.tile([128, 512], f32, tag="psq", name="psq")
                        for kc in range(8):
                            nc.tensor.matmul(
                                ps[:],
                                wq[:, kc, ft * 128 : (ft + 1) * 128],
                                xT[:, kc, tb * 512 : (tb + 1) * 512],
                                start=(kc == 0),
                                stop=(kc == 7),
                            )
                        csl = slice(tb * 512, (tb + 1) * 512)
                        if ft < 4:
                            nc.vector.tensor_scalar_add(
                                qt[:, ft, csl], ps[:], bq[:, ft : ft + 1]
                            )
                        elif ft < 8:
                            nc.vector.tensor_scalar_add(
                                kt[:, ft - 4, csl], ps[:], bq[:, ft : ft + 1]
                            )
                        else:
                            vt = sp.tile([128, 512], bf16, tag="vt", name="vt")
                            nc.vector.tensor_scalar_add(
                                vt[:], ps[:], bq[:, ft : ft + 1]
                            )
                            j = ft - 8
                            # tokens tb*512.. in v layout: kk = 4*tb..4*tb+3
                            for hh in range(2):
                                nc.sync.dma_start_transpose(
                                    v[:, 2 * j + hh, 4 * tb : 4 * tb + 4, 0:DK],
                                    vt[hh * 64 : hh * 64 + 64, :],
                                )

                def attn_block_small(j, qh):
                    q0 = qh * 512
                    av0 = psavA0.tile([128, 512], f32, tag="av0", name="av0")
                    av1 = psavA1.tile([128, 512], f32, tag="av1", name="av1")
                    pts = {}

                    def emit_st(kk):
                        st = pstA_pool.tile([128, 1024], f32, tag="st", name="st")
                        nc.tensor.matmul(
                            st[:, 0:512],
                            kt[0:64, j, kk * 128 : (kk + 1) * 128],
                            qt[0:64, j, q0 : q0 + 512],
                            start=True, stop=True,
                        )
                        nc.tensor.matmul(
                            st[:, 512:1024],
                            kt[64:128, j, kk * 128 : (kk + 1) * 128],
                            qt[64:128, j, q0 : q0 + 512],
                            start=True, stop=True,
                        )
                        pt = ptpool.tile([128, 1024], bf16, tag="pt", name="pt")
                        pts[kk] = pt
                        nc.scalar.activation(pt[:], st[:], Exp, scale=0.125)

                    def emit_av(kk):
                        pt = pts.pop(kk)
                        first, last = kk == 0, kk == 15
                        nc.tensor.matmul(
                            av0[0:65, :], v[:, 2 * j, kk, 0:65], pt[:, 0:512],
                            start=first, stop=last,
                        )
                        nc.tensor.matmul(
                            av1[0:65, :], v[:, 2 * j + 1, kk, 0:65],
                            pt[:, 512:1024],
                            start=first, stop=last,
                        )

                    for kk in range(16):
                        emit_st(kk)
                        if kk > 0:
                            emit_av(kk - 1)
                    emit_av(15)
                    evict_norm(av0, av1, j, q0, 512)

                for j in range(4):
                    qkv_tile(j)
                    qkv_tile(4 + j)
                    qkv_tile(8 + j)
                    attn_block_small(j, 0)
                    attn_block_small(j, 1)

            # ================= second half: 1024-query blocks ===============
            with (
                tc.tile_pool(name="pstB", bufs=2, space="PSUM") as pstB_pool,
                tc.tile_pool(name="psavB0", bufs=1, space="PSUM") as psavB0,
                tc.tile_pool(name="psavB1", bufs=1, space="PSUM") as psavB1,
            ):

                def attn_block_big(j):
                    av0 = psavB0.tile([128, LHALF], f32, tag="av0", name="av0")
                    av1 = psavB1.tile([128, LHALF], f32, tag="av1", name="av1")
                    pts = {}

                    def emit_st(kk):
                        stA = pstB_pool.tile([128, 1024], f32, tag="st", name="stA")
                        stB = pstB_pool.tile([128, 1024], f32, tag="st", name="stB")
                        for u in range(2):
                            q0 = LHALF + u * 512
                            nc.tensor.matmul(
                                stA[:, u * 512 : (u + 1) * 512],
                                kt[0:64, j, kk * 128 : (kk + 1) * 128],
                                qt[0:64, j, q0 : q0 + 512],
                                start=True, stop=True,
                            )
                            nc.tensor.matmul(
                                stB[:, u * 512 : (u + 1) * 512],
                                kt[64:128, j, kk * 128 : (kk + 1) * 128],
                                qt[64:128, j, q0 : q0 + 512],
                                start=True, stop=True,
                            )
                        pt = ptpool.tile([128, 2, 1024], bf16, tag="pt", name="pt")
                        pts[kk] = pt
                        nc.scalar.activation(pt[:, 0, :], stA[:], Exp, scale=0.125)
                        nc.scalar.activation(pt[:, 1, :], stB[:], Exp, scale=0.125)

                    def emit_av(kk):
                        pt = pts.pop(kk)
                        first, last = kk == 0, kk == 15
                        for u in range(2):
                            sl = slice(u * 512, (u + 1) * 512)
                            nc.tensor.matmul(
                                av0[0:65, sl], v[:, 2 * j, kk, 0:65], pt[:, 0, sl],
                                start=first, stop=last,
                            )
                            nc.tensor.matmul(
                                av1[0:65, sl], v[:, 2 * j + 1, kk, 0:65],
                                pt[:, 1, sl],
                                start=first, stop=last,
                            )

                    for kk in range(16):
                        emit_st(kk)
                        if kk > 0:
                            emit_av(kk - 1)
                    emit_av(15)
                    evict_norm(av0, av1, j, LHALF, 1024)

                def fc_chunks(qb, t8s):
                    for t8 in t8s:
                        t = qb * 8 + t8
                        pool = psavB0 if t8 % 2 == 0 else psavB1
                        tag = "av0" if t8 % 2 == 0 else "av1"
                        yp = pool.tile([128, E], f32, tag=tag, name="yp")
                        for c in range(4):
                            lhsT = onT[:, c, t * 128 : (t + 1) * 128]
                            for e2 in range(2):
                                nc.tensor.matmul(
                                    yp[:, e2 * 512 : (e2 + 1) * 512],
                                    lhsT,
                                    wfc[:, c, e2 * 512 : (e2 + 1) * 512],
                                    start=(c == 0),
                                    stop=(c == 3),
                                )
                        ys = yp_pool.tile([128, E], bf16, tag="ys", name="ys")
                        nc.vector.tensor_tensor(ys[:], yp[:], bias[:], op=ADD)
                        nc.sync.dma_start(
                            rs_in[qb][t8 * 128 : (t8 + 1) * 128, :], ys[:]
                        )

                attn_block_big(0)
                fc_chunks(0, range(8))
                nc.gpsimd.collective_compute(
                    "ReduceScatter", ADD, replica_groups=PAIRS,
                    ins=[rs_in[0].opt()], outs=[rs_out[0].opt()],
                )
                attn_block_big(1)
                nc.gpsimd.dma_start(out[0 : LHALF // 2, :], rs_out[0][:])
                attn_block_big(2)
                attn_block_big(3)
                fc_chunks(1, range(8))
                nc.gpsimd.collective_compute(
                    "ReduceScatter", ADD, replica_groups=PAIRS,
                    ins=[rs_in[1].opt()], outs=[rs_out[1].opt()],
                )
            nc.gpsimd.dma_start(out[LHALF // 2 : LHALF, :], rs_out[1][:])

    nc.finalize()
    return nc


def _prep_inputs(X, W_qkv, b_qkv, W_fc, b_fc):
    """Host-side shard + permute + cast. Returns in_maps for 8 cores."""
    X = np.asarray(X, dtype=np.float32)
    W_qkv = np.asarray(W_qkv, dtype=np.float32)
    b_qkv = np.asarray(b_qkv, dtype=np.float32)
    W_fc = np.asarray(W_fc, dtype=np.float32)
    b_fc = np.asarray(b_fc, dtype=np.float32)

    in_maps = []
    bfc_half = (0.5 * b_fc).astype(np.float32).reshape(1, E)
    for c in range(NCORES):
        b, g = divmod(c, 2)
        heads = np.arange(g * H8, (g + 1) * H8)
        cols = np.concatenate(
            [
                np.concatenate([h * 3 * DK + off + np.arange(DK) for h in heads])
                for off in (0, DK, 2 * DK)
            ]
        )
        wq_sh = W_qkv[:, cols].astype(ml_dtypes.bfloat16)
        bq_sh = b_qkv[cols].astype(np.float32).reshape(12, 128).T.copy()
        wfc_sh = W_fc[g * FO : (g + 1) * FO, :].astype(ml_dtypes.bfloat16)
        in_maps.append(
            {
                "x": np.ascontiguousarray(X[b].T).astype(ml_dtypes.bfloat16),
                "w_qkv": wq_sh,
                "b_qkv": np.ascontiguousarray(bq_sh),
                "w_fc": wfc_sh,
                "b_fc": bfc_half,
            }
        )
    return in_maps


def run_kernel(inputs, trace=False):
    if "nc" not in _CACHE:
        _CACHE["nc"] = build_nc()
    nc = _CACHE["nc"]
    in_maps = _prep_inputs(**inputs)
    res = bass_utils.run_bass_kernel_spmd(
        nc, in_maps, core_ids=list(range(NCORES)), trace=trace
    )
    Y = np.empty((B, L, E), dtype=np.float32)
    Q2 = LHALF // 2
    for c in range(NCORES):
        b, g = divmod(c, 2)
        o = res.results[c]["out"]
        Y[b, g * Q2 : (g + 1) * Q2, :] = o[0:Q2]
        Y[b, LHALF + g * Q2 : LHALF + (g + 1) * Q2, :] = o[Q2 : 2 * Q2]
    return Y, res


def kernel(X, W_qkv, b_qkv, W_fc, b_fc):
    Y, _ = run_kernel(
        dict(X=X, W_qkv=W_qkv, b_qkv=b_qkv, W_fc=W_fc, b_fc=b_fc), trace=False
    )
    return Y


if __name__ == "__main__":
    build_nc()
    print("kernel v4 compiled OK")


# revision 18
# speedup vs baseline: 1.3497x; 1.0583x over previous
"""Multi-head attention (B=4, L=2048, E=1024, H=16, DK=64) on 8 TRN2 cores.

Sharding: core c -> (batch b = c//2, head-group g = c%2 of 8 heads).
v4 schedule: first half = 512-query blocks (st ping-pong 2x[128,1024]
bufs=2 = 4 banks, av 2x[65,512] = 2 banks, dedicated 2-bank qkv psum
pool) so qkv projection tiles overlap the exp stream; second half =
1024-query blocks with per-head st half-tiles (true exp ping-pong) and
fc/collectives filling PE slack. exp runs back-to-back on ScalarE.

Self-contained: hardcodes all shapes; requires only the concourse stack.
"""

import numpy as np
import ml_dtypes

try:
    import axon_prof

    axon_prof.install()
except Exception:
    pass

import concourse.mybir as mybir
import concourse.tile as tile
from concourse import bacc
from concourse import bass_utils

B, L, E = 4, 2048, 1024
H, DK = 16, 64
H8 = 8                      # heads per core
F = H8 * 3 * DK             # qkv features per core = 1536
FO = H8 * DK                # attn-out features per core = 512
NCORES = 8
LHALF = L // 2

f32 = mybir.dt.float32
bf16 = mybir.dt.bfloat16
Exp = mybir.ActivationFunctionType.Exp
MUL = mybir.AluOpType.mult
ADD = mybir.AluOpType.add

_CACHE = {}


def build_nc():
    nc = bacc.Bacc("TRN2", target_bir_lowering=False, debug=False, num_devices=NCORES)

    x = nc.dram_tensor("x", [E, L], bf16, kind="ExternalInput")
    w_qkv = nc.dram_tensor("w_qkv", [E, F], bf16, kind="ExternalInput")
    b_qkv = nc.dram_tensor("b_qkv", [128, 12], f32, kind="ExternalInput")
    w_fc = nc.dram_tensor("w_fc", [FO, E], bf16, kind="ExternalInput")
    b_fc = nc.dram_tensor("b_fc", [1, E], f32, kind="ExternalInput")
    out = nc.dram_tensor("out", [LHALF, E], f32, kind="ExternalOutput")

    with tile.TileContext(nc) as tc:
        with (
            tc.tile_pool(name="persist", bufs=1) as pp,
            tc.tile_pool(name="work", bufs=3) as wp,
            tc.tile_pool(name="ptp", bufs=8) as ptpool,
            tc.tile_pool(name="stage", bufs=2) as sp,
            tc.tile_pool(name="srsp", bufs=1) as srsp,
            tc.tile_pool(name="ys", bufs=3) as yp_pool,
            tc.tile_pool(name="dram", bufs=1, space="DRAM") as dram,
        ):
            # ---- persistent SBUF ----
            xT = pp.tile([128, 8, L], bf16, tag="xT")          # X^T  4 MiB
            wq = pp.tile([128, 8, F], bf16, tag="wq")          # 3 MiB
            bq = pp.tile([128, 12], f32, tag="bq")
            wfc = pp.tile([128, 4, E], bf16, tag="wfc")        # 1 MiB
            bias = pp.tile([128, E], f32, tag="bias")          # 0.5 MiB
            qt = pp.tile([128, 4, L], bf16, tag="qt")          # Q^T 2 MiB
            kt = pp.tile([128, 4, L], bf16, tag="kt")          # K^T 2 MiB
            v = pp.tile([128, H8, 16, 80], bf16, tag="v")      # 2.5 MiB
            onT = pp.tile([128, 4, L], bf16, tag="onT")        # attn out^T 2 MiB

            # ---- input DMAs: xT on sync queue, wq slices on scalar queue
            for e in range(8):
                eng = nc.sync if e % 2 == 0 else nc.scalar
                eng.dma_start(xT[:, e, :], x[e * 128 : (e + 1) * 128, :])
            nc.gpsimd.dma_start(bq[:], b_qkv[:])
            nc.gpsimd.dma_start(wfc[:], w_fc.rearrange("(c p) e -> p c e", p=128))
            bfc_row = pp.tile([1, E], f32, tag="bfc_row")
            nc.gpsimd.dma_start(bfc_row[:], b_fc[:])
            nc.gpsimd.partition_broadcast(bias[:], bfc_row[:])
            nc.vector.memset(v[:, :, :, 64:65], 1.0)
            FT_ORDER = []
            for j in range(4):
                FT_ORDER += [j, 4 + j, 8 + j]
            for ft in FT_ORDER:
                nc.scalar.dma_start(
                    wq[:, :, ft * 128 : (ft + 1) * 128],
                    w_qkv.rearrange("(c p) f -> p c f", p=128)[
                        :, :, ft * 128 : (ft + 1) * 128
                    ],
                )
            # preload the exp table set during the DMA phase
            warm = pp.tile([128, 16], f32, tag="warm")
            nc.vector.memset(warm[:], 0.0)
            nc.scalar.activation(warm[:], warm[:], Exp)

            rs_in = [
                dram.tile([LHALF, E], bf16, name=f"rs_in{i}", tag=f"rs_in{i}")
                for i in range(2)
            ]
            rs_out = [
                dram.tile([LHALF // 2, E], bf16, name="rs_out0", tag="rs_out0"),
                dram.tile([LHALF // 4, E], bf16, name="rs_out2a", tag="rs_out2a"),
                dram.tile([LHALF // 4, E], bf16, name="rs_out2b", tag="rs_out2b"),
            ]
            PAIRS = [[0, 1], [2, 3], [4, 5], [6, 7]]

            def evict_norm(av0, av1, j, q0, qn):
                # unnormalized out^T + deferred normalization
                qsl = slice(q0, q0 + qn)
                nc.vector.tensor_copy(onT[0:64, j, qsl], av0[0:64, :])
                tmp = wp.tile([64, 1024], bf16, tag="tmp", name="tmp")
                nc.vector.tensor_copy(tmp[0:64, 0:qn], av1[0:64, :])
                srs = srsp.tile([128, 4096], f32, tag="srs", name="srs")
                nc.vector.tensor_copy(srs[64:65, 0:qn], av0[64:65, :])
                nc.vector.tensor_copy(srs[64:65, qn : 2 * qn], av1[64:65, :])
                nc.sync.dma_start(onT[64:128, j, qsl], tmp[0:64, 0:qn])
                nc.sync.dma_start(srs[0:1, 0 : 2 * qn], srs[64:65, 0 : 2 * qn])
                nc.gpsimd.partition_broadcast(
                    srs[:, 2048 : 2048 + 2 * qn], srs[0:1, 0 : 2 * qn]
                )
                nc.vector.reciprocal_approx_fast(
                    srs[:, 2048 : 2048 + 2 * qn], srs[:, 2048 : 2048 + 2 * qn]
                )
                nc.vector.tensor_tensor(
                    onT[0:64, j, qsl], onT[0:64, j, qsl],
                    srs[0:64, 2048 : 2048 + qn], op=MUL,
                )
                nc.vector.tensor_tensor(
                    onT[64:128, j, qsl], onT[64:128, j, qsl],
                    srs[64:128, 2048 + qn : 2048 + 2 * qn], op=MUL,
                )

            # ================= first half: 512-query blocks =================
            with (
                tc.tile_pool(name="pstA", bufs=2, space="PSUM") as pstA_pool,
                tc.tile_pool(name="psavA0", bufs=1, space="PSUM") as psavA0,
                tc.tile_pool(name="psavA1", bufs=1, space="PSUM") as psavA1,
                tc.tile_pool(name="psqkv", bufs=2, space="PSUM") as psqkv,
            ):

                def qkv_tile(ft):
                    for tb in range(4):
                        ps = psqkv.tile([128, 512], f32, tag="psq", name="psq")
                        for kc in range(8):
                            nc.tensor.matmul(
                                ps[:],
                                wq[:, kc, ft * 128 : (ft + 1) * 128],
                                xT[:, kc, tb * 512 : (tb + 1) * 512],
                                start=(kc == 0),
                                stop=(kc == 7),
                            )
                        csl = slice(tb * 512, (tb + 1) * 512)
                        if ft < 4:
                            nc.vector.tensor_scalar_add(
                                qt[:, ft, csl], ps[:], bq[:, ft : ft + 1]
                            )
                        elif ft < 8:
                            nc.vector.tensor_scalar_add(
                                kt[:, ft - 4, csl], ps[:], bq[:, ft : ft + 1]
                            )
                        else:
                            vt = sp.tile([128, 512], bf16, tag="vt", name="vt")
                            nc.vector.tensor_scalar_add(
                                vt[:], ps[:], bq[:, ft : ft + 1]
                            )
                            j = ft - 8
                            # tokens tb*512.. in v layout: kk = 4*tb..4*tb+3
                            for hh in range(2):
                                nc.sync.dma_start_transpose(
                                    v[:, 2 * j + hh, 4 * tb : 4 * tb + 4, 0:DK],
                                    vt[hh * 64 : hh * 64 + 64, :],
                                )

                def attn_block_small(j, qh):
                    q0 = qh * 512
                    av0 = psavA0.tile([128, 512], f32, tag="av0", name="av0")
                    av1 = psavA1.tile([128, 512], f32, tag="av1", name="av1")
                    pts = {}

                    def emit_st(kk):
                        st = pstA_pool.tile([128, 1024], f32, tag="st", name="st")
                        nc.tensor.matmul(
                            st[:, 0:512],
                            kt[0:64, j, kk * 128 : (kk + 1) * 128],
                            qt[0:64, j, q0 : q0 + 512],
                            start=True, stop=True,
                        )
                        nc.tensor.matmul(
                            st[:, 512:1024],
                            kt[64:128, j, kk * 128 : (kk + 1) * 128],
                            qt[64:128, j, q0 : q0 + 512],
                            start=True, stop=True,
                        )
                        pt = ptpool.tile([128, 1024], bf16, tag="pt", name="pt")
                        pts[kk] = pt
                        nc.scalar.activation(pt[:], st[:], Exp, scale=0.125)

                    def emit_av(kk):
                        pt = pts.pop(kk)
                        first, last = kk == 0, kk == 15
                        nc.tensor.matmul(
                            av0[0:65, :], v[:, 2 * j, kk, 0:65], pt[:, 0:512],
                            start=first, stop=last,
                        )
                        nc.tensor.matmul(
                            av1[0:65, :], v[:, 2 * j + 1, kk, 0:65],
                            pt[:, 512:1024],
                            start=first, stop=last,
                        )

                    for kk in range(16):
                        emit_st(kk)
                        if kk > 0:
                            emit_av(kk - 1)
                    emit_av(15)
                    evict_norm(av0, av1, j, q0, 512)

                for j in range(4):
                    qkv_tile(j)
                    qkv_tile(4 + j)
                    qkv_tile(8 + j)
                    attn_block_small(j, 0)
                    attn_block_small(j, 1)

            # ================= second half: 1024-query blocks ===============
            with (
                tc.tile_pool(name="pstB", bufs=2, space="PSUM") as pstB_pool,
                tc.tile_pool(name="psavB0", bufs=1, space="PSUM") as psavB0,
                tc.tile_pool(name="psavB1", bufs=1, space="PSUM") as psavB1,
            ):

                def attn_block_big(j):
                    av0 = psavB0.tile([128, LHALF], f32, tag="av0", name="av0")
                    av1 = psavB1.tile([128, LHALF], f32, tag="av1", name="av1")
                    pts = {}

                    def emit_st(kk):
                        stA = pstB_pool.tile([128, 1024], f32, tag="st", name="stA")
                        stB = pstB_pool.tile([128, 1024], f32, tag="st", name="stB")
                        for u in range(2):
                            q0 = LHALF + u * 512
                            nc.tensor.matmul(
                                stA[:, u * 512 : (u + 1) * 512],
                                kt[0:64, j, kk * 128 : (kk + 1) * 128],
                                qt[0:64, j, q0 : q0 + 512],
                                start=True, stop=True,
                            )
                            nc.tensor.matmul(
                                stB[:, u * 512 : (u + 1) * 512],
                                kt[64:128, j, kk * 128 : (kk + 1) * 128],
                                qt[64:128, j, q0 : q0 + 512],
                                start=True, stop=True,
                            )
                        pt = ptpool.tile([128, 2, 1024], bf16, tag="pt", name="pt")
                        pts[kk] = pt
                        nc.scalar.activation(pt[:, 0, :], stA[:], Exp, scale=0.125)
                        nc.scalar.activation(pt[:, 1, :], stB[:], Exp, scale=0.125)

                    def emit_av(kk):
                        pt = pts.pop(kk)
                        first, last = kk == 0, kk == 15
                        for u in range(2):
                            sl = slice(u * 512, (u + 1) * 512)
                            nc.tensor.matmul(
                                av0[0:65, sl], v[:, 2 * j, kk, 0:65], pt[:, 0, sl],
                                start=first, stop=last,
                            )
                            nc.tensor.matmul(
                                av1[0:65, sl], v[:, 2 * j + 1, kk, 0:65],
                                pt[:, 1, sl],
                                start=first, stop=last,
                            )

                    for kk in range(16):
                        emit_st(kk)
                        if kk > 0:
                            emit_av(kk - 1)
                    emit_av(15)
                    evict_norm(av0, av1, j, LHALF, 1024)

                def fc_chunks(qb, t8s):
                    for t8 in t8s:
                        t = qb * 8 + t8
                        pool = psavB0 if t8 % 2 == 0 else psavB1
                        tag = "av0" if t8 % 2 == 0 else "av1"
                        yp = pool.tile([128, E], f32, tag=tag, name="yp")
                        for c in range(4):
                            lhsT = onT[:, c, t * 128 : (t + 1) * 128]
                            for e2 in range(2):
                                nc.tensor.matmul(
                                    yp[:, e2 * 512 : (e2 + 1) * 512],
                                    lhsT,
                                    wfc[:, c, e2 * 512 : (e2 + 1) * 512],
                                    start=(c == 0),
                                    stop=(c == 3),
                                )
                        ys = yp_pool.tile([128, E], bf16, tag="ys", name="ys")
                        nc.vector.tensor_tensor(ys[:], yp[:], bias[:], op=ADD)
                        nc.sync.dma_start(
                            rs_in[qb][t8 * 128 : (t8 + 1) * 128, :], ys[:]
                        )

                def attn_block_small2(j, q0):
                    # 2 heads x 512 queries, second-half pools
                    av0 = psavB0.tile([128, 512], f32, tag="av0", name="av0")
                    av1 = psavB1.tile([128, 512], f32, tag="av1", name="av1")
                    pts = {}

                    def emit_st(kk):
                        st = pstB_pool.tile([128, 1024], f32, tag="st", name="st")
                        nc.tensor.matmul(
                            st[:, 0:512],
                            kt[0:64, j, kk * 128 : (kk + 1) * 128],
                            qt[0:64, j, q0 : q0 + 512],
                            start=True, stop=True,
                        )
                        nc.tensor.matmul(
                            st[:, 512:1024],
                            kt[64:128, j, kk * 128 : (kk + 1) * 128],
                            qt[64:128, j, q0 : q0 + 512],
                            start=True, stop=True,
                        )
                        pt = ptpool.tile([128, 1024], bf16, tag="pt", name="pt")
                        pts[kk] = pt
                        nc.scalar.activation(pt[:], st[:], Exp, scale=0.125)

                    def emit_av(kk):
                        pt = pts.pop(kk)
                        first, last = kk == 0, kk == 15
                        nc.tensor.matmul(
                            av0[0:65, :], v[:, 2 * j, kk, 0:65], pt[:, 0:512],
                            start=first, stop=last,
                        )
                        nc.tensor.matmul(
                            av1[0:65, :], v[:, 2 * j + 1, kk, 0:65],
                            pt[:, 512:1024],
                            start=first, stop=last,
                        )

                    for kk in range(16):
                        emit_st(kk)
                        if kk > 0:
                            emit_av(kk - 1)
                    emit_av(15)
                    evict_norm(av0, av1, j, q0, 512)

                attn_block_big(0)
                fc_chunks(0, range(8))
                nc.gpsimd.collective_compute(
                    "ReduceScatter", ADD, replica_groups=PAIRS,
                    ins=[rs_in[0].opt()], outs=[rs_out[0].opt()],
                )
                attn_block_big(1)
                nc.gpsimd.dma_start(out[0 : LHALF // 2, :], rs_out[0][:])
                attn_block_big(2)
                attn_block_small2(3, LHALF)
                fc_chunks(1, range(4))
                nc.gpsimd.collective_compute(
                    "ReduceScatter", ADD, replica_groups=PAIRS,
                    ins=[rs_in[1][0 : LHALF // 2, :].opt()],
                    outs=[rs_out[1].opt()],
                )
                attn_block_small2(3, LHALF + 512)
                nc.gpsimd.dma_start(
                    out[LHALF // 2 : 3 * LHALF // 4, :], rs_out[1][:]
                )
                fc_chunks(1, range(4, 8))
                nc.gpsimd.collective_compute(
                    "ReduceScatter", ADD, replica_groups=PAIRS,
                    ins=[rs_in[1][LHALF // 2 :, :].opt()],
                    outs=[rs_out[2].opt()],
                )
            nc.gpsimd.dma_start(out[3 * LHALF // 4 :, :], rs_out[2][:])

    nc.finalize()
    return nc


def _prep_inputs(X, W_qkv, b_qkv, W_fc, b_fc):
    """Host-side shard + permute + cast. Returns in_maps for 8 cores."""
    X = np.asarray(X, dtype=np.float32)
    W_qkv = np.asarray(W_qkv, dtype=np.float32)
    b_qkv = np.asarray(b_qkv, dtype=np.float32)
    W_fc = np.asarray(W_fc, dtype=np.float32)
    b_fc = np.asarray(b_fc, dtype=np.float32)

    in_maps = []
    bfc_half = (0.5 * b_fc).astype(np.float32).reshape(1, E)
    for c in range(NCORES):
        b, g = divmod(c, 2)
        heads = np.arange(g * H8, (g + 1) * H8)
        cols = np.concatenate(
            [
                np.concatenate([h * 3 * DK + off + np.arange(DK) for h in heads])
                for off in (0, DK, 2 * DK)
            ]
        )
        wq_sh = W_qkv[:, cols].astype(ml_dtypes.bfloat16)
        bq_sh = b_qkv[cols].astype(np.float32).reshape(12, 128).T.copy()
        wfc_sh = W_fc[g * FO : (g + 1) * FO, :].astype(ml_dtypes.bfloat16)
        in_maps.append(
            {
                "x": np.ascontiguousarray(X[b].T).astype(ml_dtypes.bfloat16),
                "w_qkv": wq_sh,
                "b_qkv": np.ascontiguousarray(bq_sh),
                "w_fc": wfc_sh,
                "b_fc": bfc_half,
            }
        )
    return in_maps


def run_kernel(inputs, trace=False):
    if "nc" not in _CACHE:
        _CACHE["nc"] = build_nc()
    nc = _CACHE["nc"]
    in_maps = _prep_inputs(**inputs)
    res = bass_utils.run_bass_kernel_spmd(
        nc, in_maps, core_ids=list(range(NCORES)), trace=trace
    )
    Y = np.empty((B, L, E), dtype=np.float32)
    Q2 = LHALF // 2
    Q4 = LHALF // 4
    for c in range(NCORES):
        b, g = divmod(c, 2)
        o = res.results[c]["out"]
        Y[b, g * Q2 : (g + 1) * Q2, :] = o[0:Q2]
        Y[b, LHALF + g * Q4 : LHALF + (g + 1) * Q4, :] = o[Q2 : Q2 + Q4]
        Y[b, LHALF + Q2 + g * Q4 : LHALF + Q2 + (g + 1) * Q4, :] = o[Q2 + Q4 :]
    return Y, res


def kernel(X, W_qkv, b_qkv, W_fc, b_fc):
    Y, _ = run_kernel(
        dict(X=X, W_qkv=W_qkv, b_qkv=b_qkv, W_fc=W_fc, b_fc=b_fc), trace=False
    )
    return Y


if __name__ == "__main__":
    build_nc()
    print("kernel v4 compiled OK")
